# revision 40
# baseline (speedup 1.0000x reference)
"""Causal self-attention (GPT-style, 12 heads, C=768) on 8 TRN2 NeuronCores.

Sharding: core c -> (batch b = c//2, head-group g = c%2 of 6 heads).
Each core computes qkv projection for its 6 heads, causal attention, and a
partial output projection (its 384 rows of w_proj). Host sums the two
partial projections per batch (row-parallel tensor parallelism) and adds
the folded bias (b_proj + bv@w_proj; the v-bias commutes through softmax).

Layouts chosen so no on-device transposes are needed:
  - x is transposed on host -> xT [C, T]
  - qkv matmul produces qT/kT directly ([head-pair d, T]); V in natural [T, d]
  - scores computed transposed: sT[j, i] = K Q^T via lhsT=kT, rhs=qT
  - softmax denominator via a ones-column in V (AV psum row 64 = sum_j exp)
  - outT = av * (1/S); 1/S on DVE (approx-fast), partition-broadcast on GPSIMD

Pipeline: one unit per (pair, j-tile): score pair (row-tiled concurrent,
K=64 at PE rows 0/64) -> one exp covering both heads -> AV pair. The shared
score psum tile makes the h0/h1 exp gating symmetric. qkv chains for chunk
ic+1 and projection blocks for chunk ic-1 are interleaved into the attention
stream as fillers so the PE queue always has exp-independent work.
"""

import numpy as np

import concourse.bass as bass
import concourse.mybir as mybir
import concourse.tile as tile
from concourse import bacc
from concourse import bass_utils

f32 = mybir.dt.float32
bf16 = mybir.dt.bfloat16
AF = mybir.ActivationFunctionType
ALU = mybir.AluOpType

N_HEAD = 12
N_EMBD = 768
B_FULL = 4
T_FULL = 2048
N_CORES = 8
SCALE = float(N_EMBD) ** -0.5

TRACE = False
LAST_RESULT = None
_NC_CACHE = {}


def build_nc(T=T_FULL):
    """Build the per-core Bass program. All 8 cores run this same program
    on different input data."""
    C = N_EMBD            # 768
    NP = 3                # head pairs
    KT = C // 128         # 6 k-tiles for the projections
    NIC = T // 512        # i-chunks (512 queries each)
    NJT = T // 128        # j-tiles (128 keys each)
    pairs = [(0, slice(0, 64)), (1, slice(64, 128))]

    # Pin softmax Exp to one activation-table set so the table-load pass
    # emits a single load.
    import concourse.bacc as _bacc_mod
    from concourse.hw_specs import get_activation_tables as _orig_gat

    def _pinned_gat(arch):
        tabs = {k: set(v) for k, v in _orig_gat(arch).items()}
        for name, fns in tabs.items():
            if name != "natural_log_exp_and_others":
                fns.discard(AF.Exp)
                fns.discard(AF.Ln)
        return tabs

    nc = bacc.Bacc("TRN2", target_bir_lowering=False, debug=False)

    xT_d = nc.dram_tensor("xT", [C, T], bf16, kind="ExternalInput")
    wqk_d = nc.dram_tensor("wqk", [C, 768], bf16, kind="ExternalInput")
    wv_d = nc.dram_tensor("wv", [C, 384], bf16, kind="ExternalInput")
    wp_d = nc.dram_tensor("wp", [384, C], bf16, kind="ExternalInput")
    bqk_d = nc.dram_tensor("bqk", [128, 6], f32, kind="ExternalInput")
    mask_d = nc.dram_tensor("mask", [128, 2, 128], bf16, kind="ExternalInput")
    y_d = nc.dram_tensor("y", [T, C], bf16, kind="ExternalOutput")

    with tile.TileContext(nc) as tc:
        with (
            tc.tile_pool(name="const", bufs=1) as constp,
            tc.tile_pool(name="xt", bufs=4) as xtp,
            tc.tile_pool(name="qk", bufs=1) as qkp,
            tc.tile_pool(name="vs", bufs=16) as vsp,
            tc.tile_pool(name="es", bufs=12) as esp,
            tc.tile_pool(name="ot", bufs=1) as otp,
            tc.tile_pool(name="ys", bufs=6) as ysp,
            tc.tile_pool(name="rs", bufs=1) as rsp,
            tc.tile_pool(name="psg", bufs=2, space="PSUM") as psgp,
            tc.tile_pool(name="pav", bufs=2, space="PSUM") as pavp,
        ):
            # ---------------- startup loads -------------------------------
            # Per-k-tile DMAs across the three DMA-capable queues; the first
            # qkv matmul needs only wqk k0 + x k0 (~0.35MB), not the full
            # 2.5MB weight+x load.
            qs = [nc.sync, nc.scalar, nc.gpsimd]
            wqk_t = constp.tile([128, KT, 768], bf16, tag="wqk")
            wqk_src = wqk_d.ap().rearrange("(k p) c -> p k c", p=128)
            xts0_t = xtp.tile([128, KT, 512], bf16, tag="xt")
            xsrc0 = xT_d.ap().rearrange("(k p) t -> p k t", p=128)[:, :, 0:512]
            # k-tiles arrive in the order the first qkv chain consumes them
            for k in range(KT):
                qs[(2 * k) % 3].dma_start(wqk_t[:, k:k + 1, :],
                                          wqk_src[:, k:k + 1, :])
                qs[(2 * k + 1) % 3].dma_start(xts0_t[:, k:k + 1, :],
                                              xsrc0[:, k:k + 1, :])
            wqk = [wqk_t[:, k, :] for k in range(KT)]
            bqk_t = constp.tile([128, 6], f32, tag="bqk")
            nc.sync.dma_start(bqk_t[:], bqk_d.ap()[:])
            bqk = [bqk_t[:, m:m + 1] for m in range(6)]
            wv_t = constp.tile([128, KT, 384], bf16, tag="wv")
            wv_src = wv_d.ap().rearrange("(k p) c -> p k c", p=128)
            for k in range(0, KT, 2):
                qs[(k // 2) % 3].dma_start(
                    wv_t[:, k:k + 2, :], wv_src[:, k:k + 2, :])
            wv = [wv_t[:, k, :] for k in range(KT)]

            # ---------------- persistent SBUF tensors ---------------------
            qT = [qkp.tile([128, T], bf16, tag=f"qT{p}", name=f"qT{p}")
                  for p in range(NP)]
            kT = [qkp.tile([128, T], bf16, tag=f"kT{p}", name=f"kT{p}")
                  for p in range(NP)]
            v = [vsp.tile([128, 6, 65], bf16, tag="v", name=f"v{j}")
                 for j in range(NJT)]
            for j in range(NJT):
                nc.vector.memset(v[j][:, :, 64:65], 1.0)
            outT = [otp.tile([128, T], bf16, tag=f"outT{p}", name=f"outT{p}")
                    for p in range(NP)]

            # ---------------- qkv projection chains -----------------------
            def emit_qkv_dma(tci):
                if tci == 0:
                    return [xts0_t[:, k, :] for k in range(KT)]
                ts512 = slice(512 * tci, 512 * (tci + 1))
                xts_t = xtp.tile([128, KT, 512], bf16, tag="xt")
                xsrc = xT_d.ap().rearrange("(k p) t -> p k t",
                                           p=128)[:, :, ts512]
                nc.gpsimd.dma_start(xts_t[:, 0:3, :], xsrc[:, 0:3, :])
                nc.sync.dma_start(xts_t[:, 3:6, :], xsrc[:, 3:6, :])
                return [xts_t[:, k, :] for k in range(KT)]

            def qkv_qk_chain(tci, xts, m):
                ts512 = slice(512 * tci, 512 * (tci + 1))
                ps = psgp.tile([128, 512], f32, tag="sg", name="psqk")
                for k in range(KT):
                    nc.tensor.matmul(ps[:], wqk[k][:, 128 * m:128 * (m + 1)],
                                     xts[k],
                                     start=(k == 0), stop=(k == KT - 1))
                dest = qT[m] if m < 3 else kT[m - 3]
                nc.vector.tensor_scalar_add(dest[:, ts512], ps[:], bqk[m])

            def qkv_v_chain(tci, xts, tsub):
                jt = 4 * tci + tsub
                ps = psgp.tile([128, 384], f32, tag="sg", name="psv")
                for k in range(KT):
                    nc.tensor.matmul(
                        ps[:],
                        xts[k][:, 128 * tsub:128 * (tsub + 1)],
                        wv[k],
                        start=(k == 0), stop=(k == KT - 1))
                nc.vector.tensor_copy(
                    v[jt][:, :, 0:64],
                    ps[:].rearrange("p (h d) -> p h d", h=6))

            def emit_late_consts():
                msk = constp.tile([128, 2, 128], bf16, tag="msk")
                nc.sync.dma_start(msk[:], mask_d.ap()[:])
                wp_t = constp.tile([128, NP, 768], bf16, tag="wp")
                wp_src = wp_d.ap().rearrange("(m p) c -> p m c", p=128)
                for m in range(NP):
                    qs[m % 3].dma_start(
                        wp_t[:, m:m + 1, :], wp_src[:, m:m + 1, :])
                return msk, [wp_t[:, m, :] for m in range(NP)]

            # ---------------- projection block ----------------------------
            def proj_block(ic, tsub, n):
                t0 = 512 * ic + 128 * tsub
                nsl = slice(384 * n, 384 * (n + 1))
                ysb = ysp.tile([128, 384], bf16, tag="y")
                yp = psgp.tile([128, 384], f32, tag="sg", name="yp")
                for mp in range(NP):
                    nc.tensor.matmul(
                        yp[:], outT[mp][:, t0:t0 + 128], wp[mp][:, nsl],
                        start=(mp == 0), stop=(mp == NP - 1))
                nc.vector.tensor_copy(ysb[:], yp[:])
                nc.sync.dma_start(y_d.ap()[t0:t0 + 128, nsl], ysb[:])

            # ---------------- attention chunk with fillers ----------------
            # AV runs one unit behind its scores (software pipeline): the PE
            # executes [scores(jt), av(jt-1), filler?] per unit so av never
            # head-of-line-blocks on the exp(jt) latency. The pending-AV
            # state carries across pair and chunk boundaries so the last AV
            # of a pair is flushed under the NEXT pair's first scores, and
            # the pair's normalize is emitted at that point.
            pend = {"av": None, "fin": None}

            def flush_pending():
                if pend["av"] is not None:
                    pend["av"]()
                    pend["av"] = None
                if pend["fin"] is not None:
                    pend["fin"]()
                    pend["fin"] = None

            def emit_normalize(p, ic, av, piecewise_proj=False):
                isl = slice(512 * ic, 512 * (ic + 1))
                sf = rsp.tile([1, 2, 512], f32, tag="sf", bufs=2)
                for h, dsl in pairs:
                    nc.vector.tensor_copy(sf[:, h, :], av[h][64:65, :])
                rf = rsp.tile([1, 2, 512], f32, tag="rf", bufs=2)
                nc.vector.reciprocal_approx_fast(rf[:], sf[:])
                rbs = rsp.tile([64, 2, 512], f32, tag="rbs", bufs=2)
                nc.gpsimd.partition_broadcast(rbs[:], rf[:])
                if not piecewise_proj:
                    for h, dsl in pairs:
                        nc.vector.tensor_tensor(outT[p][dsl, isl],
                                                av[h][0:64, :], rbs[:, h, :],
                                                op=ALU.mult)
                    return
                # kernel tail: normalize 128-query pieces and launch the
                # final projection blocks as each piece lands
                for tsub in range(4):
                    csl = slice(128 * tsub, 128 * (tsub + 1))
                    for h, dsl in pairs:
                        nc.vector.tensor_tensor(
                            outT[p][dsl, 512 * ic + 128 * tsub:
                                    512 * ic + 128 * (tsub + 1)],
                            av[h][0:64, csl], rbs[:, h, csl],
                            op=ALU.mult)
                    proj_block(ic, tsub, 0)
                    proj_block(ic, tsub, 1)

            def emit_normalize_tail(p, ic, av):
                # fully piecewise: reciprocal/broadcast/multiply/projection
                # pipelined per 128-query piece so the PE starts the final
                # projection ~3us earlier than a whole-pair normalize
                for tsub in range(4):
                    csl = slice(128 * tsub, 128 * (tsub + 1))
                    sfp = rsp.tile([1, 2, 128], f32, tag="sfp", bufs=4)
                    for h, dsl in pairs:
                        nc.vector.tensor_copy(sfp[:, h, :],
                                              av[h][64:65, csl])
                    rfp = rsp.tile([1, 2, 128], f32, tag="rfp", bufs=4)
                    nc.vector.reciprocal_approx_fast(rfp[:], sfp[:])
                    rbp = rsp.tile([64, 2, 128], f32, tag="rbp", bufs=4)
                    nc.gpsimd.partition_broadcast(rbp[:], rfp[:])
                    for h, dsl in pairs:
                        nc.vector.tensor_tensor(
                            outT[p][dsl, 512 * ic + 128 * tsub:
                                    512 * ic + 128 * (tsub + 1)],
                            av[h][0:64, csl], rbp[:, h, :],
                            op=ALU.mult)
                    proj_block(ic, tsub, 0)
                    proj_block(ic, tsub, 1)

            def emit_attn_chunk(ic, fillers):
                njt = 4 * ic + 4
                nunits = NP * njt
                # hold back two fillers for the chunk boundary: they give
                # the PE exp-independent work while ScalarE drains the last
                # pair's exps, covering the next chunk's first-score wait
                nres = 2 if len(fillers) > 4 else 0
                fillers = fillers[:len(fillers) - nres] + fillers[
                    len(fillers) - nres:]
                npaced = len(fillers) - nres
                unit = 0
                fdone = 0
                for p in range(NP):
                    av = {h: pavp.tile([65, 512], f32, tag=f"av{h}",
                                       name=f"av{p}{h}", bufs=2)
                          for h, _ in pairs}
                    for jt in range(njt):
                        d = jt - 4 * ic
                        i0 = 128 * d if d >= 0 else 0
                        w = 512 - i0
                        sg = psgp.tile([128, 2, 512], f32, tag="sg",
                                       name="sg")
                        for h, dsl in pairs:
                            nc.tensor.matmul(
                                sg[:, h, 0:w],
                                kT[p][dsl, 128 * jt:128 * (jt + 1)],
                                qT[p][dsl, 512 * ic + i0:512 * (ic + 1)],
                                start=True, stop=True,
                                tile_position=(64 * h, 0))
                        et = esp.tile([128, 2, 512], bf16, tag="e")
                        nc.scalar.activation(et[:, :, 0:w], sg[:, :, 0:w],
                                             AF.Exp, scale=SCALE)
                        if d >= 0:
                            # leading 128 query-cols straddle the diagonal
                            nc.vector.tensor_tensor(
                                et[:, :, 0:128], et[:, :, 0:128], msk[:],
                                op=ALU.mult)
                        flush_pending()

                        def mk_av(p=p, av=av, jt=jt, i0=i0, w=w, et=et,
                                  njt=njt):
                            def go():
                                for h, dsl in pairs:
                                    hl = 2 * p + h
                                    nc.tensor.matmul(
                                        av[h][:, i0:512], v[jt][:, hl, :],
                                        et[:, h, 0:w],
                                        start=(jt == 0),
                                        stop=(jt == njt - 1))
                            return go
                        pend["av"] = mk_av()
                        if jt == njt - 1:
                            if ic == NIC - 1 and p == NP - 1:
                                pend["fin"] = (
                                    lambda p=p, ic=ic, av=av:
                                    emit_normalize_tail(p, ic, av))
                            else:
                                pend["fin"] = (
                                    lambda p=p, ic=ic, av=av:
                                    emit_normalize(p, ic, av))
                        unit += 1
                        while fdone < npaced and \
                                fdone * nunits < unit * npaced:
                            fillers[fdone]()
                            fdone += 1
                while fdone < len(fillers):
                    fillers[fdone]()
                    fdone += 1

            # ---------------- emission topology ---------------------------
            xts = emit_qkv_dma(0)
            for m in range(6):
                qkv_qk_chain(0, xts, m)
            for tsub in range(4):
                qkv_v_chain(0, xts, tsub)
            msk, wp = emit_late_consts()
            # v-chains 1..3 of each chunk are deferred into that chunk's own
            # attention stream (their v[jt] is consumed mid-pair-0 at the
            # earliest), giving the filler-starved late chunks PE work.
            deferred = []
            for ic in range(NIC):
                fillers = list(deferred)
                deferred = []
                if ic + 1 < NIC:
                    xts = emit_qkv_dma(ic + 1)
                    fillers += [
                        (lambda t=ic + 1, x=xts, m=m: qkv_qk_chain(t, x, m))
                        for m in range(6)]
                    fillers += [
                        (lambda t=ic + 1, x=xts: qkv_v_chain(t, x, 0))]
                    deferred = [
                        (lambda t=ic + 1, x=xts, s=s: qkv_v_chain(t, x, s))
                        for s in range(1, 4)]
                if ic >= 1:
                    fillers += [
                        (lambda c=ic - 1, s=s, n=n: proj_block(c, s, n))
                        for s in range(4) for n in range(2)]
                emit_attn_chunk(ic, fillers)
            flush_pending()   # final AV + piecewise normalize/proj tail

    _bacc_mod.get_activation_tables = _pinned_gat
    try:
        nc.compile()
    finally:
        _bacc_mod.get_activation_tables = _orig_gat
    return nc


def make_in_maps(x, w_attn, b_attn, w_proj, b_proj, T=T_FULL):
    import ml_dtypes
    bf = ml_dtypes.bfloat16
    x = np.asarray(x, np.float32)
    w_attn = np.asarray(w_attn, np.float32)
    b_attn = np.asarray(b_attn, np.float32)
    w_proj = np.asarray(w_proj, np.float32)
    B = x.shape[0]

    # tril mask for the leading 128-column diagonal sub-block (two identical
    # copies so one op covers both heads of the shared exp tile)
    mask = np.broadcast_to(
        (np.arange(128)[:, None, None] <= np.arange(128)[None, None, :]),
        (128, 2, 128)).astype(np.float32)

    in_maps = []
    for c in range(N_CORES):
        b, g = (c // 2) % B, c % 2
        q0, k0, v0 = 384 * g, 768 + 384 * g, 1536 + 384 * g
        wqk = np.concatenate(
            [w_attn[:, q0:q0 + 384], w_attn[:, k0:k0 + 384]], axis=1)
        bqk = np.concatenate(
            [b_attn[q0:q0 + 384], b_attn[k0:k0 + 384]])
        in_maps.append({
            "xT": np.ascontiguousarray(x[b].T).astype(bf),
            "wqk": np.ascontiguousarray(wqk).astype(bf),
            "wv": np.ascontiguousarray(w_attn[:, v0:v0 + 384]).astype(bf),
            "wp": np.ascontiguousarray(w_proj[384 * g:384 * (g + 1), :]).astype(bf),
            "bqk": np.ascontiguousarray(bqk.reshape(6, 128).T),
            "mask": np.ascontiguousarray(mask).astype(bf),
        })
    return in_maps


def kernel(x, w_attn, b_attn, w_proj, b_proj):
    global LAST_RESULT
    if "nc" not in _NC_CACHE:
        _NC_CACHE["nc"] = build_nc(T_FULL)
    nc = _NC_CACHE["nc"]
    in_maps = make_in_maps(x, w_attn, b_attn, w_proj, b_proj)
    res = bass_utils.run_bass_kernel_spmd(
        nc, in_maps, core_ids=list(range(N_CORES)), trace=TRACE)
    LAST_RESULT = res
    B, T, C = np.asarray(x).shape
    # host-folded bias: v-bias commutes through softmax (weights sum to 1),
    # so y += (b_attn_v @ w_proj) + b_proj once per row.
    b_attn = np.asarray(b_attn, np.float32)
    hbias = b_attn[2 * C:3 * C] @ np.asarray(w_proj, np.float32) \
        + np.asarray(b_proj, np.float32)
    y = np.empty((B, T, C), np.float32)
    for b in range(B):
        y[b] = res.results[2 * b]["y"].astype(np.float32) \
            + res.results[2 * b + 1]["y"].astype(np.float32) + hbias
    return y


# revision 41
# speedup vs baseline: 1.0036x; 1.0036x over previous
"""Causal self-attention (GPT-style, 12 heads, C=768) on 8 TRN2 NeuronCores.

Sharding: core c -> (batch b = c//2, head-group g = c%2 of 6 heads).
Each core computes qkv projection for its 6 heads, causal attention, and a
partial output projection (its 384 rows of w_proj). Host sums the two
partial projections per batch (row-parallel tensor parallelism) and adds
the folded bias (b_proj + bv@w_proj; the v-bias commutes through softmax).

Layouts chosen so no on-device transposes are needed:
  - x is transposed on host -> xT [C, T]
  - qkv matmul produces qT/kT directly ([head-pair d, T]); V in natural [T, d]
  - scores computed transposed: sT[j, i] = K Q^T via lhsT=kT, rhs=qT
  - softmax denominator via a ones-column in V (AV psum row 64 = sum_j exp)
  - outT = av * (1/S); 1/S on DVE (approx-fast), partition-broadcast on GPSIMD

Pipeline: one unit per (pair, j-tile): score pair (row-tiled concurrent,
K=64 at PE rows 0/64) -> one exp covering both heads -> AV pair. The shared
score psum tile makes the h0/h1 exp gating symmetric. qkv chains for chunk
ic+1 and projection blocks for chunk ic-1 are interleaved into the attention
stream as fillers so the PE queue always has exp-independent work.
"""

import numpy as np

import concourse.bass as bass
import concourse.mybir as mybir
import concourse.tile as tile
from concourse import bacc
from concourse import bass_utils

f32 = mybir.dt.float32
bf16 = mybir.dt.bfloat16
AF = mybir.ActivationFunctionType
ALU = mybir.AluOpType

N_HEAD = 12
N_EMBD = 768
B_FULL = 4
T_FULL = 2048
N_CORES = 8
SCALE = float(N_EMBD) ** -0.5

TRACE = False
LAST_RESULT = None
_NC_CACHE = {}


def build_nc(T=T_FULL):
    """Build the per-core Bass program. All 8 cores run this same program
    on different input data."""
    C = N_EMBD            # 768
    NP = 3                # head pairs
    KT = C // 128         # 6 k-tiles for the projections
    NIC = T // 512        # i-chunks (512 queries each)
    NJT = T // 128        # j-tiles (128 keys each)
    pairs = [(0, slice(0, 64)), (1, slice(64, 128))]

    # Pin softmax Exp to one activation-table set so the table-load pass
    # emits a single load.
    import concourse.bacc as _bacc_mod
    from concourse.hw_specs import get_activation_tables as _orig_gat

    def _pinned_gat(arch):
        tabs = {k: set(v) for k, v in _orig_gat(arch).items()}
        for name, fns in tabs.items():
            if name != "natural_log_exp_and_others":
                fns.discard(AF.Exp)
                fns.discard(AF.Ln)
        return tabs

    nc = bacc.Bacc("TRN2", target_bir_lowering=False, debug=False)

    xT_d = nc.dram_tensor("xT", [C, T], bf16, kind="ExternalInput")
    wqk_d = nc.dram_tensor("wqk", [C, 768], bf16, kind="ExternalInput")
    wv_d = nc.dram_tensor("wv", [C, 384], bf16, kind="ExternalInput")
    wp_d = nc.dram_tensor("wp", [384, C], bf16, kind="ExternalInput")
    bqk_d = nc.dram_tensor("bqk", [128, 6], f32, kind="ExternalInput")
    mask_d = nc.dram_tensor("mask", [128, 2, 128], bf16, kind="ExternalInput")
    y_d = nc.dram_tensor("y", [T, C], bf16, kind="ExternalOutput")

    with tile.TileContext(nc) as tc:
        with (
            tc.tile_pool(name="const", bufs=1) as constp,
            tc.tile_pool(name="xt", bufs=3) as xtp,
            tc.tile_pool(name="qk", bufs=1) as qkp,
            tc.tile_pool(name="vs", bufs=16) as vsp,
            tc.tile_pool(name="es", bufs=8) as esp,
            tc.tile_pool(name="ot", bufs=1) as otp,
            tc.tile_pool(name="ys", bufs=4) as ysp,
            tc.tile_pool(name="rs", bufs=1) as rsp,
            tc.tile_pool(name="psg", bufs=2, space="PSUM") as psgp,
            tc.tile_pool(name="pav", bufs=2, space="PSUM") as pavp,
        ):
            # ---------------- startup loads -------------------------------
            # Per-k-tile DMAs across the three DMA-capable queues; the first
            # qkv matmul needs only wqk k0 + x k0 (~0.35MB), not the full
            # 2.5MB weight+x load.
            qs = [nc.sync, nc.scalar, nc.gpsimd]
            wqk_t = constp.tile([128, KT, 768], bf16, tag="wqk")
            wqk_src = wqk_d.ap().rearrange("(k p) c -> p k c", p=128)
            xts0_t = xtp.tile([128, KT, 512], bf16, tag="xt")
            xsrc0 = xT_d.ap().rearrange("(k p) t -> p k t", p=128)[:, :, 0:512]
            # k-tiles arrive in the order the first qkv chain consumes them
            for k in range(KT):
                qs[(2 * k) % 3].dma_start(wqk_t[:, k:k + 1, :],
                                          wqk_src[:, k:k + 1, :])
                qs[(2 * k + 1) % 3].dma_start(xts0_t[:, k:k + 1, :],
                                              xsrc0[:, k:k + 1, :])
            wqk = [wqk_t[:, k, :] for k in range(KT)]
            bqk_t = constp.tile([128, 6], f32, tag="bqk")
            nc.sync.dma_start(bqk_t[:], bqk_d.ap()[:])
            bqk = [bqk_t[:, m:m + 1] for m in range(6)]
            wv_t = constp.tile([128, KT, 384], bf16, tag="wv")
            wv_src = wv_d.ap().rearrange("(k p) c -> p k c", p=128)
            for k in range(0, KT, 2):
                qs[(k // 2) % 3].dma_start(
                    wv_t[:, k:k + 2, :], wv_src[:, k:k + 2, :])
            wv = [wv_t[:, k, :] for k in range(KT)]

            # ---------------- persistent SBUF tensors ---------------------
            qT = [qkp.tile([128, T], bf16, tag=f"qT{p}", name=f"qT{p}")
                  for p in range(NP)]
            kT = [qkp.tile([128, T], bf16, tag=f"kT{p}", name=f"kT{p}")
                  for p in range(NP)]
            v = [vsp.tile([128, 6, 65], bf16, tag="v", name=f"v{j}")
                 for j in range(NJT)]
            for j in range(NJT):
                nc.vector.memset(v[j][:, :, 64:65], 1.0)
            outT = [otp.tile([128, T], bf16, tag=f"outT{p}", name=f"outT{p}")
                    for p in range(NP)]

            # ---------------- qkv projection chains -----------------------
            def emit_qkv_dma(tci):
                if tci == 0:
                    return [xts0_t[:, k, :] for k in range(KT)]
                ts512 = slice(512 * tci, 512 * (tci + 1))
                xts_t = xtp.tile([128, KT, 512], bf16, tag="xt")
                xsrc = xT_d.ap().rearrange("(k p) t -> p k t",
                                           p=128)[:, :, ts512]
                nc.gpsimd.dma_start(xts_t[:, 0:3, :], xsrc[:, 0:3, :])
                nc.sync.dma_start(xts_t[:, 3:6, :], xsrc[:, 3:6, :])
                return [xts_t[:, k, :] for k in range(KT)]

            def qkv_qk_chain(tci, xts, m):
                ts512 = slice(512 * tci, 512 * (tci + 1))
                ps = psgp.tile([128, 512], f32, tag="sg", name="psqk")
                for k in range(KT):
                    nc.tensor.matmul(ps[:], wqk[k][:, 128 * m:128 * (m + 1)],
                                     xts[k],
                                     start=(k == 0), stop=(k == KT - 1))
                dest = qT[m] if m < 3 else kT[m - 3]
                nc.vector.tensor_scalar_add(dest[:, ts512], ps[:], bqk[m])

            def qkv_v_chain(tci, xts, tsub):
                jt = 4 * tci + tsub
                ps = psgp.tile([128, 384], f32, tag="sg", name="psv")
                for k in range(KT):
                    nc.tensor.matmul(
                        ps[:],
                        xts[k][:, 128 * tsub:128 * (tsub + 1)],
                        wv[k],
                        start=(k == 0), stop=(k == KT - 1))
                nc.vector.tensor_copy(
                    v[jt][:, :, 0:64],
                    ps[:].rearrange("p (h d) -> p h d", h=6))

            def emit_late_consts():
                msk = constp.tile([128, 2, 128], bf16, tag="msk")
                nc.sync.dma_start(msk[:], mask_d.ap()[:])
                wp_t = constp.tile([128, NP, 768], bf16, tag="wp")
                wp_src = wp_d.ap().rearrange("(m p) c -> p m c", p=128)
                for m in range(NP):
                    qs[m % 3].dma_start(
                        wp_t[:, m:m + 1, :], wp_src[:, m:m + 1, :])
                return msk, [wp_t[:, m, :] for m in range(NP)]

            # ---------------- projection block ----------------------------
            def proj_block(ic, tsub, n):
                t0 = 512 * ic + 128 * tsub
                nsl = slice(384 * n, 384 * (n + 1))
                ysb = ysp.tile([128, 384], bf16, tag="y")
                yp = psgp.tile([128, 384], f32, tag="sg", name="yp")
                for mp in range(NP):
                    nc.tensor.matmul(
                        yp[:], outT[mp][:, t0:t0 + 128], wp[mp][:, nsl],
                        start=(mp == 0), stop=(mp == NP - 1))
                nc.vector.tensor_copy(ysb[:], yp[:])
                nc.sync.dma_start(y_d.ap()[t0:t0 + 128, nsl], ysb[:])

            # ---------------- attention chunk with fillers ----------------
            # AV runs one unit behind its scores (software pipeline): the PE
            # executes [scores(jt), av(jt-1), filler?] per unit so av never
            # head-of-line-blocks on the exp(jt) latency. The pending-AV
            # state carries across pair and chunk boundaries so the last AV
            # of a pair is flushed under the NEXT pair's first scores, and
            # the pair's normalize is emitted at that point.
            pend = {"av": None, "fin": None}

            def flush_pending():
                if pend["av"] is not None:
                    pend["av"]()
                    pend["av"] = None
                if pend["fin"] is not None:
                    pend["fin"]()
                    pend["fin"] = None

            def emit_normalize(p, ic, av, piecewise_proj=False):
                isl = slice(512 * ic, 512 * (ic + 1))
                sf = rsp.tile([1, 2, 512], f32, tag="sf", bufs=2)
                for h, dsl in pairs:
                    nc.vector.tensor_copy(sf[:, h, :], av[h][64:65, :])
                rf = rsp.tile([1, 2, 512], f32, tag="rf", bufs=2)
                nc.vector.reciprocal_approx_fast(rf[:], sf[:])
                rbs = rsp.tile([64, 2, 512], f32, tag="rbs", bufs=2)
                nc.gpsimd.partition_broadcast(rbs[:], rf[:])
                if not piecewise_proj:
                    for h, dsl in pairs:
                        nc.vector.tensor_tensor(outT[p][dsl, isl],
                                                av[h][0:64, :], rbs[:, h, :],
                                                op=ALU.mult)
                    return
                # kernel tail: normalize 128-query pieces and launch the
                # final projection blocks as each piece lands
                for tsub in range(4):
                    csl = slice(128 * tsub, 128 * (tsub + 1))
                    for h, dsl in pairs:
                        nc.vector.tensor_tensor(
                            outT[p][dsl, 512 * ic + 128 * tsub:
                                    512 * ic + 128 * (tsub + 1)],
                            av[h][0:64, csl], rbs[:, h, csl],
                            op=ALU.mult)
                    proj_block(ic, tsub, 0)
                    proj_block(ic, tsub, 1)

            def emit_normalize_tail(p, ic, av):
                # fully piecewise: reciprocal/broadcast/multiply/projection
                # pipelined per 128-query piece so the PE starts the final
                # projection ~3us earlier than a whole-pair normalize
                for tsub in range(4):
                    csl = slice(128 * tsub, 128 * (tsub + 1))
                    sfp = rsp.tile([1, 2, 128], f32, tag="sfp", bufs=4)
                    for h, dsl in pairs:
                        nc.vector.tensor_copy(sfp[:, h, :],
                                              av[h][64:65, csl])
                    rfp = rsp.tile([1, 2, 128], f32, tag="rfp", bufs=4)
                    nc.vector.reciprocal_approx_fast(rfp[:], sfp[:])
                    rbp = rsp.tile([64, 2, 128], f32, tag="rbp", bufs=4)
                    nc.gpsimd.partition_broadcast(rbp[:], rfp[:])
                    for h, dsl in pairs:
                        nc.vector.tensor_tensor(
                            outT[p][dsl, 512 * ic + 128 * tsub:
                                    512 * ic + 128 * (tsub + 1)],
                            av[h][0:64, csl], rbp[:, h, :],
                            op=ALU.mult)
                    proj_block(ic, tsub, 0)
                    proj_block(ic, tsub, 1)

            def emit_attn_chunk(ic, fillers):
                njt = 4 * ic + 4
                nunits = NP * njt
                # hold back two fillers for the chunk boundary: they give
                # the PE exp-independent work while ScalarE drains the last
                # pair's exps, covering the next chunk's first-score wait
                nres = 2 if len(fillers) > 4 else 0
                fillers = fillers[:len(fillers) - nres] + fillers[
                    len(fillers) - nres:]
                npaced = len(fillers) - nres
                unit = 0
                fdone = 0
                for p in range(NP):
                    av = {h: pavp.tile([65, 512], f32, tag=f"av{h}",
                                       name=f"av{p}{h}", bufs=2)
                          for h, _ in pairs}
                    for jt in range(njt):
                        d = jt - 4 * ic
                        i0 = 128 * d if d >= 0 else 0
                        w = 512 - i0
                        sg = psgp.tile([128, 2, 512], f32, tag="sg",
                                       name="sg")
                        for h, dsl in pairs:
                            nc.tensor.matmul(
                                sg[:, h, 0:w],
                                kT[p][dsl, 128 * jt:128 * (jt + 1)],
                                qT[p][dsl, 512 * ic + i0:512 * (ic + 1)],
                                start=True, stop=True,
                                tile_position=(64 * h, 0))
                        et = esp.tile([128, 2, 512], bf16, tag="e")
                        nc.scalar.activation(et[:, :, 0:w], sg[:, :, 0:w],
                                             AF.Exp, scale=SCALE)
                        if d >= 0:
                            # leading 128 query-cols straddle the diagonal
                            nc.vector.tensor_tensor(
                                et[:, :, 0:128], et[:, :, 0:128], msk[:],
                                op=ALU.mult)
                        flush_pending()

                        def mk_av(p=p, av=av, jt=jt, i0=i0, w=w, et=et,
                                  njt=njt):
                            def go():
                                for h, dsl in pairs:
                                    hl = 2 * p + h
                                    nc.tensor.matmul(
                                        av[h][:, i0:512], v[jt][:, hl, :],
                                        et[:, h, 0:w],
                                        start=(jt == 0),
                                        stop=(jt == njt - 1))
                            return go
                        pend["av"] = mk_av()
                        if jt == njt - 1:
                            if ic == NIC - 1 and p == NP - 1:
                                pend["fin"] = (
                                    lambda p=p, ic=ic, av=av:
                                    emit_normalize_tail(p, ic, av))
                            else:
                                pend["fin"] = (
                                    lambda p=p, ic=ic, av=av:
                                    emit_normalize(p, ic, av))
                        unit += 1
                        while fdone < npaced and \
                                fdone * nunits < unit * npaced:
                            fillers[fdone]()
                            fdone += 1
                while fdone < len(fillers):
                    fillers[fdone]()
                    fdone += 1

            # ---------------- emission topology ---------------------------
            xts = emit_qkv_dma(0)
            for m in range(6):
                qkv_qk_chain(0, xts, m)
            for tsub in range(4):
                qkv_v_chain(0, xts, tsub)
            msk, wp = emit_late_consts()
            # v-chains 1..3 of each chunk are deferred into that chunk's own
            # attention stream (their v[jt] is consumed mid-pair-0 at the
            # earliest), giving the filler-starved late chunks PE work.
            deferred = []
            for ic in range(NIC):
                fillers = list(deferred)
                deferred = []
                if ic + 1 < NIC:
                    xts = emit_qkv_dma(ic + 1)
                    fillers += [
                        (lambda t=ic + 1, x=xts, m=m: qkv_qk_chain(t, x, m))
                        for m in range(6)]
                    fillers += [
                        (lambda t=ic + 1, x=xts: qkv_v_chain(t, x, 0))]
                    deferred = [
                        (lambda t=ic + 1, x=xts, s=s: qkv_v_chain(t, x, s))
                        for s in range(1, 4)]
                if ic >= 1:
                    fillers += [
                        (lambda c=ic - 1, s=s, n=n: proj_block(c, s, n))
                        for s in range(4) for n in range(2)]
                emit_attn_chunk(ic, fillers)
            flush_pending()   # final AV + piecewise normalize/proj tail

    _bacc_mod.get_activation_tables = _pinned_gat
    try:
        nc.compile()
    finally:
        _bacc_mod.get_activation_tables = _orig_gat
    return nc


def make_in_maps(x, w_attn, b_attn, w_proj, b_proj, T=T_FULL):
    import ml_dtypes
    bf = ml_dtypes.bfloat16
    x = np.asarray(x, np.float32)
    w_attn = np.asarray(w_attn, np.float32)
    b_attn = np.asarray(b_attn, np.float32)
    w_proj = np.asarray(w_proj, np.float32)
    B = x.shape[0]

    # tril mask for the leading 128-column diagonal sub-block (two identical
    # copies so one op covers both heads of the shared exp tile)
    mask = np.broadcast_to(
        (np.arange(128)[:, None, None] <= np.arange(128)[None, None, :]),
        (128, 2, 128)).astype(np.float32)

    in_maps = []
    for c in range(N_CORES):
        b, g = (c // 2) % B, c % 2
        q0, k0, v0 = 384 * g, 768 + 384 * g, 1536 + 384 * g
        wqk = np.concatenate(
            [w_attn[:, q0:q0 + 384], w_attn[:, k0:k0 + 384]], axis=1)
        bqk = np.concatenate(
            [b_attn[q0:q0 + 384], b_attn[k0:k0 + 384]])
        in_maps.append({
            "xT": np.ascontiguousarray(x[b].T).astype(bf),
            "wqk": np.ascontiguousarray(wqk).astype(bf),
            "wv": np.ascontiguousarray(w_attn[:, v0:v0 + 384]).astype(bf),
            "wp": np.ascontiguousarray(w_proj[384 * g:384 * (g + 1), :]).astype(bf),
            "bqk": np.ascontiguousarray(bqk.reshape(6, 128).T),
            "mask": np.ascontiguousarray(mask).astype(bf),
        })
    return in_maps


def kernel(x, w_attn, b_attn, w_proj, b_proj):
    global LAST_RESULT
    if "nc" not in _NC_CACHE:
        _NC_CACHE["nc"] = build_nc(T_FULL)
    nc = _NC_CACHE["nc"]
    in_maps = make_in_maps(x, w_attn, b_attn, w_proj, b_proj)
    res = bass_utils.run_bass_kernel_spmd(
        nc, in_maps, core_ids=list(range(N_CORES)), trace=TRACE)
    LAST_RESULT = res
    B, T, C = np.asarray(x).shape
    # host-folded bias: v-bias commutes through softmax (weights sum to 1),
    # so y += (b_attn_v @ w_proj) + b_proj once per row.
    b_attn = np.asarray(b_attn, np.float32)
    hbias = b_attn[2 * C:3 * C] @ np.asarray(w_proj, np.float32) \
        + np.asarray(b_proj, np.float32)
    y = np.empty((B, T, C), np.float32)
    for b in range(B):
        y[b] = res.results[2 * b]["y"].astype(np.float32) \
            + res.results[2 * b + 1]["y"].astype(np.float32) + hbias
    return y


# revision 42
# speedup vs baseline: 1.0137x; 1.0101x over previous
"""Causal self-attention (GPT-style, 12 heads, C=768) on 8 TRN2 NeuronCores.

Sharding: core c -> (batch b = c//2, head-group g = c%2 of 6 heads).
Each core computes qkv projection for its 6 heads, causal attention, and a
partial output projection (its 384 rows of w_proj). Host sums the two
partial projections per batch (row-parallel tensor parallelism) and adds
the folded bias (b_proj + bv@w_proj; the v-bias commutes through softmax).

Layouts chosen so no on-device transposes are needed:
  - x is transposed on host -> xT [C, T]
  - qkv matmul produces qT/kT directly ([head-pair d, T]); V in natural [T, d]
  - scores computed transposed: sT[j, i] = K Q^T via lhsT=kT, rhs=qT
  - softmax denominator via a ones-column in V (AV psum row 64 = sum_j exp)
  - outT = av * (1/S); 1/S on DVE (approx-fast), partition-broadcast on GPSIMD

Pipeline: one unit per (pair, j-tile): score pair (row-tiled concurrent,
K=64 at PE rows 0/64) -> one exp covering both heads -> AV pair. The shared
score psum tile makes the h0/h1 exp gating symmetric. qkv chains for chunk
ic+1 and projection blocks for chunk ic-1 are interleaved into the attention
stream as fillers so the PE queue always has exp-independent work.
"""

import numpy as np

import concourse.bass as bass
import concourse.mybir as mybir
import concourse.tile as tile
from concourse import bacc
from concourse import bass_utils

f32 = mybir.dt.float32
bf16 = mybir.dt.bfloat16
AF = mybir.ActivationFunctionType
ALU = mybir.AluOpType

N_HEAD = 12
N_EMBD = 768
B_FULL = 4
T_FULL = 2048
N_CORES = 8
SCALE = float(N_EMBD) ** -0.5

TRACE = False
LAST_RESULT = None
_NC_CACHE = {}


def build_nc(T=T_FULL):
    """Build the per-core Bass program. All 8 cores run this same program
    on different input data."""
    C = N_EMBD            # 768
    NP = 3                # head pairs
    KT = C // 128         # 6 k-tiles for the projections
    NIC = T // 512        # i-chunks (512 queries each)
    NJT = T // 128        # j-tiles (128 keys each)
    pairs = [(0, slice(0, 64)), (1, slice(64, 128))]

    # Pin softmax Exp to one activation-table set so the table-load pass
    # emits a single load.
    import concourse.bacc as _bacc_mod
    from concourse.hw_specs import get_activation_tables as _orig_gat

    def _pinned_gat(arch):
        tabs = {k: set(v) for k, v in _orig_gat(arch).items()}
        for name, fns in tabs.items():
            if name != "natural_log_exp_and_others":
                fns.discard(AF.Exp)
                fns.discard(AF.Ln)
        return tabs

    nc = bacc.Bacc("TRN2", target_bir_lowering=False, debug=False)

    xT_d = nc.dram_tensor("xT", [C, T], bf16, kind="ExternalInput")
    wqk_d = nc.dram_tensor("wqk", [C, 768], bf16, kind="ExternalInput")
    wv_d = nc.dram_tensor("wv", [C, 384], bf16, kind="ExternalInput")
    wp_d = nc.dram_tensor("wp", [384, C], bf16, kind="ExternalInput")
    bqk_d = nc.dram_tensor("bqk", [128, 6], f32, kind="ExternalInput")
    mask_d = nc.dram_tensor("mask", [128, 2, 128], bf16, kind="ExternalInput")
    y_d = nc.dram_tensor("y", [T, C], bf16, kind="ExternalOutput")

    with tile.TileContext(nc) as tc:
        with (
            tc.tile_pool(name="const", bufs=1) as constp,
            tc.tile_pool(name="xt", bufs=3) as xtp,
            tc.tile_pool(name="qk", bufs=1) as qkp,
            tc.tile_pool(name="vs", bufs=16) as vsp,
            tc.tile_pool(name="es", bufs=8) as esp,
            tc.tile_pool(name="ot", bufs=1) as otp,
            tc.tile_pool(name="ys", bufs=4) as ysp,
            tc.tile_pool(name="rs", bufs=1) as rsp,
            tc.tile_pool(name="psg", bufs=2, space="PSUM") as psgp,
            tc.tile_pool(name="pav", bufs=2, space="PSUM") as pavp,
        ):
            # ---------------- startup loads -------------------------------
            # Per-k-tile DMAs across the three DMA-capable queues; the first
            # qkv matmul needs only wqk k0 + x k0 (~0.35MB), not the full
            # 2.5MB weight+x load.
            qs = [nc.sync, nc.scalar, nc.gpsimd]
            wqk_t = constp.tile([128, KT, 768], bf16, tag="wqk")
            wqk_src = wqk_d.ap().rearrange("(k p) c -> p k c", p=128)
            xts0_t = xtp.tile([128, KT, 512], bf16, tag="xt")
            xsrc0 = xT_d.ap().rearrange("(k p) t -> p k t", p=128)[:, :, 0:512]
            # k-tiles arrive in the order the first qkv chain consumes them
            for k in range(KT):
                qs[(2 * k) % 3].dma_start(wqk_t[:, k:k + 1, :],
                                          wqk_src[:, k:k + 1, :])
                qs[(2 * k + 1) % 3].dma_start(xts0_t[:, k:k + 1, :],
                                              xsrc0[:, k:k + 1, :])
            wqk = [wqk_t[:, k, :] for k in range(KT)]
            bqk_t = constp.tile([128, 6], f32, tag="bqk")
            nc.sync.dma_start(bqk_t[:], bqk_d.ap()[:])
            bqk = [bqk_t[:, m:m + 1] for m in range(6)]
            wv_t = constp.tile([128, KT, 384], bf16, tag="wv")
            wv_src = wv_d.ap().rearrange("(k p) c -> p k c", p=128)
            for k in range(0, KT, 2):
                qs[(k // 2) % 3].dma_start(
                    wv_t[:, k:k + 2, :], wv_src[:, k:k + 2, :])
            wv = [wv_t[:, k, :] for k in range(KT)]

            # ---------------- persistent SBUF tensors ---------------------
            qT = [qkp.tile([128, T], bf16, tag=f"qT{p}", name=f"qT{p}")
                  for p in range(NP)]
            kT = [qkp.tile([128, T], bf16, tag=f"kT{p}", name=f"kT{p}")
                  for p in range(NP)]
            v = [vsp.tile([128, 6, 65], bf16, tag="v", name=f"v{j}")
                 for j in range(NJT)]
            for j in range(NJT):
                nc.vector.memset(v[j][:, :, 64:65], 1.0)
            outT = [otp.tile([128, T], bf16, tag=f"outT{p}", name=f"outT{p}")
                    for p in range(NP)]

            # ---------------- qkv projection chains -----------------------
            def emit_qkv_dma(tci):
                if tci == 0:
                    return [xts0_t[:, k, :] for k in range(KT)]
                ts512 = slice(512 * tci, 512 * (tci + 1))
                xts_t = xtp.tile([128, KT, 512], bf16, tag="xt")
                xsrc = xT_d.ap().rearrange("(k p) t -> p k t",
                                           p=128)[:, :, ts512]
                nc.gpsimd.dma_start(xts_t[:, 0:3, :], xsrc[:, 0:3, :])
                nc.sync.dma_start(xts_t[:, 3:6, :], xsrc[:, 3:6, :])
                return [xts_t[:, k, :] for k in range(KT)]

            def qkv_qk_chain(tci, xts, m):
                ts512 = slice(512 * tci, 512 * (tci + 1))
                ps = psgp.tile([128, 512], f32, tag="sg", name="psqk")
                for k in range(KT):
                    nc.tensor.matmul(ps[:], wqk[k][:, 128 * m:128 * (m + 1)],
                                     xts[k],
                                     start=(k == 0), stop=(k == KT - 1))
                dest = qT[m] if m < 3 else kT[m - 3]
                nc.vector.tensor_scalar_add(dest[:, ts512], ps[:], bqk[m])

            def qkv_v_chain(tci, xts, tsub):
                jt = 4 * tci + tsub
                ps = psgp.tile([128, 384], f32, tag="sg", name="psv")
                for k in range(KT):
                    nc.tensor.matmul(
                        ps[:],
                        xts[k][:, 128 * tsub:128 * (tsub + 1)],
                        wv[k],
                        start=(k == 0), stop=(k == KT - 1))
                nc.vector.tensor_copy(
                    v[jt][:, :, 0:64],
                    ps[:].rearrange("p (h d) -> p h d", h=6))

            def emit_late_consts():
                msk = constp.tile([128, 2, 128], bf16, tag="msk")
                nc.sync.dma_start(msk[:], mask_d.ap()[:])
                wp_t = constp.tile([128, NP, 768], bf16, tag="wp")
                wp_src = wp_d.ap().rearrange("(m p) c -> p m c", p=128)
                for m in range(NP):
                    qs[m % 3].dma_start(
                        wp_t[:, m:m + 1, :], wp_src[:, m:m + 1, :])
                return msk, [wp_t[:, m, :] for m in range(NP)]

            # ---------------- projection block ----------------------------
            def proj_block(ic, tsub, n):
                t0 = 512 * ic + 128 * tsub
                nsl = slice(384 * n, 384 * (n + 1))
                ysb = ysp.tile([128, 384], bf16, tag="y")
                yp = psgp.tile([128, 384], f32, tag="sg", name="yp")
                for mp in range(NP):
                    nc.tensor.matmul(
                        yp[:], outT[mp][:, t0:t0 + 128], wp[mp][:, nsl],
                        start=(mp == 0), stop=(mp == NP - 1))
                nc.vector.tensor_copy(ysb[:], yp[:])
                nc.sync.dma_start(y_d.ap()[t0:t0 + 128, nsl], ysb[:])

            # ---------------- attention chunk with fillers ----------------
            # AV runs one unit behind its scores (software pipeline): the PE
            # executes [scores(jt), av(jt-1), filler?] per unit so av never
            # head-of-line-blocks on the exp(jt) latency. The pending-AV
            # state carries across pair and chunk boundaries so the last AV
            # of a pair is flushed under the NEXT pair's first scores, and
            # the pair's normalize is emitted at that point.
            pend = {"av": None, "fin": None}

            def flush_pending():
                if pend["av"] is not None:
                    pend["av"]()
                    pend["av"] = None
                if pend["fin"] is not None:
                    pend["fin"]()
                    pend["fin"] = None

            def emit_normalize(p, ic, av, piecewise_proj=False):
                isl = slice(512 * ic, 512 * (ic + 1))
                sf = rsp.tile([1, 2, 512], f32, tag="sf", bufs=2)
                for h, dsl in pairs:
                    nc.vector.tensor_copy(sf[:, h, :], av[h][64:65, :])
                rf = rsp.tile([1, 2, 512], f32, tag="rf", bufs=2)
                nc.vector.reciprocal_approx_fast(rf[:], sf[:])
                rbs = rsp.tile([64, 2, 512], f32, tag="rbs", bufs=2)
                nc.gpsimd.partition_broadcast(rbs[:], rf[:])
                if not piecewise_proj:
                    for h, dsl in pairs:
                        nc.vector.tensor_tensor(outT[p][dsl, isl],
                                                av[h][0:64, :], rbs[:, h, :],
                                                op=ALU.mult)
                    return
                # kernel tail: normalize 128-query pieces and launch the
                # final projection blocks as each piece lands
                for tsub in range(4):
                    csl = slice(128 * tsub, 128 * (tsub + 1))
                    for h, dsl in pairs:
                        nc.vector.tensor_tensor(
                            outT[p][dsl, 512 * ic + 128 * tsub:
                                    512 * ic + 128 * (tsub + 1)],
                            av[h][0:64, csl], rbs[:, h, csl],
                            op=ALU.mult)
                    proj_block(ic, tsub, 0)
                    proj_block(ic, tsub, 1)

            def emit_normalize_tail(p, ic, av):
                # fully piecewise: reciprocal/broadcast/multiply/projection
                # pipelined per 128-query piece so the PE starts the final
                # projection ~3us earlier than a whole-pair normalize
                for tsub in range(4):
                    csl = slice(128 * tsub, 128 * (tsub + 1))
                    sfp = rsp.tile([1, 2, 128], f32, tag="sfp", bufs=4)
                    for h, dsl in pairs:
                        nc.vector.tensor_copy(sfp[:, h, :],
                                              av[h][64:65, csl])
                    rfp = rsp.tile([1, 2, 128], f32, tag="rfp", bufs=4)
                    nc.vector.reciprocal_approx_fast(rfp[:], sfp[:])
                    rbp = rsp.tile([64, 2, 128], f32, tag="rbp", bufs=4)
                    nc.gpsimd.partition_broadcast(rbp[:], rfp[:])
                    for h, dsl in pairs:
                        nc.vector.tensor_tensor(
                            outT[p][dsl, 512 * ic + 128 * tsub:
                                    512 * ic + 128 * (tsub + 1)],
                            av[h][0:64, csl], rbp[:, h, :],
                            op=ALU.mult)
                    proj_block(ic, tsub, 0)
                    proj_block(ic, tsub, 1)

            def emit_attn_chunk(ic, fillers):
                njt = 4 * ic + 4
                # hold back two fillers for the chunk boundary: they give
                # the PE exp-independent work while ScalarE drains the last
                # pair's exps, covering the next chunk's first-score wait
                nres = 2 if len(fillers) > 4 else 0
                fillers = fillers[:len(fillers) - nres] + fillers[
                    len(fillers) - nres:]
                npaced = len(fillers) - nres
                unit = 0
                fdone = 0
                # work items: one per j-tile, except the two narrowest
                # diagonal tiles (w=256, w=128) share one item -> one exp
                # instruction and one pipeline unit instead of two
                items = []
                for jt in range(njt):
                    d = jt - 4 * ic
                    i0 = 128 * d if d >= 0 else 0
                    items.append([(jt, i0, 512 - i0)])
                items = items[:-2] + [items[-2] + items[-1]]
                nunits = NP * len(items)
                for p in range(NP):
                    av = {h: pavp.tile([65, 512], f32, tag=f"av{h}",
                                       name=f"av{p}{h}", bufs=2)
                          for h, _ in pairs}
                    for it in items:
                        segs = []
                        c0 = 0
                        for (jt, i0, w) in it:
                            segs.append((jt, i0, w, c0))
                            c0 += w
                        wtot = c0
                        sg = psgp.tile([128, 2, 512], f32, tag="sg",
                                       name="sg")
                        for (jt, i0, w, c0s) in segs:
                            for h, dsl in pairs:
                                nc.tensor.matmul(
                                    sg[:, h, c0s:c0s + w],
                                    kT[p][dsl, 128 * jt:128 * (jt + 1)],
                                    qT[p][dsl, 512 * ic + i0:512 * (ic + 1)],
                                    start=True, stop=True,
                                    tile_position=(64 * h, 0))
                        et = esp.tile([128, 2, 512], bf16, tag="e")
                        nc.scalar.activation(et[:, :, 0:wtot],
                                             sg[:, :, 0:wtot],
                                             AF.Exp, scale=SCALE)
                        for (jt, i0, w, c0s) in segs:
                            if jt - 4 * ic >= 0:
                                # leading 128 query-cols straddle the diag
                                nc.vector.tensor_tensor(
                                    et[:, :, c0s:c0s + 128],
                                    et[:, :, c0s:c0s + 128], msk[:],
                                    op=ALU.mult)
                        flush_pending()

                        def mk_av(p=p, av=av, segs=segs, et=et, njt=njt):
                            def go():
                                for (jt, i0, w, c0s) in segs:
                                    for h, dsl in pairs:
                                        hl = 2 * p + h
                                        nc.tensor.matmul(
                                            av[h][:, i0:512],
                                            v[jt][:, hl, :],
                                            et[:, h, c0s:c0s + w],
                                            start=(jt == 0),
                                            stop=(jt == njt - 1))
                            return go
                        pend["av"] = mk_av()
                        if segs[-1][0] == njt - 1:
                            if ic == NIC - 1 and p == NP - 1:
                                pend["fin"] = (
                                    lambda p=p, ic=ic, av=av:
                                    emit_normalize_tail(p, ic, av))
                            else:
                                pend["fin"] = (
                                    lambda p=p, ic=ic, av=av:
                                    emit_normalize(p, ic, av))
                        unit += 1
                        while fdone < npaced and \
                                fdone * nunits < unit * npaced:
                            fillers[fdone]()
                            fdone += 1
                while fdone < len(fillers):
                    fillers[fdone]()
                    fdone += 1

            # ---------------- emission topology ---------------------------
            xts = emit_qkv_dma(0)
            for m in range(6):
                qkv_qk_chain(0, xts, m)
            for tsub in range(4):
                qkv_v_chain(0, xts, tsub)
            msk, wp = emit_late_consts()
            # v-chains 1..3 of each chunk are deferred into that chunk's own
            # attention stream (their v[jt] is consumed mid-pair-0 at the
            # earliest), giving the filler-starved late chunks PE work.
            deferred = []
            for ic in range(NIC):
                fillers = list(deferred)
                deferred = []
                if ic + 1 < NIC:
                    xts = emit_qkv_dma(ic + 1)
                    fillers += [
                        (lambda t=ic + 1, x=xts, m=m: qkv_qk_chain(t, x, m))
                        for m in range(6)]
                    fillers += [
                        (lambda t=ic + 1, x=xts: qkv_v_chain(t, x, 0))]
                    deferred = [
                        (lambda t=ic + 1, x=xts, s=s: qkv_v_chain(t, x, s))
                        for s in range(1, 4)]
                if ic >= 1:
                    fillers += [
                        (lambda c=ic - 1, s=s, n=n: proj_block(c, s, n))
                        for s in range(4) for n in range(2)]
                emit_attn_chunk(ic, fillers)
            flush_pending()   # final AV + piecewise normalize/proj tail

    _bacc_mod.get_activation_tables = _pinned_gat
    try:
        nc.compile()
    finally:
        _bacc_mod.get_activation_tables = _orig_gat
    return nc


def make_in_maps(x, w_attn, b_attn, w_proj, b_proj, T=T_FULL):
    import ml_dtypes
    bf = ml_dtypes.bfloat16
    x = np.asarray(x, np.float32)
    w_attn = np.asarray(w_attn, np.float32)
    b_attn = np.asarray(b_attn, np.float32)
    w_proj = np.asarray(w_proj, np.float32)
    B = x.shape[0]

    # tril mask for the leading 128-column diagonal sub-block (two identical
    # copies so one op covers both heads of the shared exp tile)
    mask = np.broadcast_to(
        (np.arange(128)[:, None, None] <= np.arange(128)[None, None, :]),
        (128, 2, 128)).astype(np.float32)

    in_maps = []
    for c in range(N_CORES):
        b, g = (c // 2) % B, c % 2
        q0, k0, v0 = 384 * g, 768 + 384 * g, 1536 + 384 * g
        wqk = np.concatenate(
            [w_attn[:, q0:q0 + 384], w_attn[:, k0:k0 + 384]], axis=1)
        bqk = np.concatenate(
            [b_attn[q0:q0 + 384], b_attn[k0:k0 + 384]])
        in_maps.append({
            "xT": np.ascontiguousarray(x[b].T).astype(bf),
            "wqk": np.ascontiguousarray(wqk).astype(bf),
            "wv": np.ascontiguousarray(w_attn[:, v0:v0 + 384]).astype(bf),
            "wp": np.ascontiguousarray(w_proj[384 * g:384 * (g + 1), :]).astype(bf),
            "bqk": np.ascontiguousarray(bqk.reshape(6, 128).T),
            "mask": np.ascontiguousarray(mask).astype(bf),
        })
    return in_maps


def kernel(x, w_attn, b_attn, w_proj, b_proj):
    global LAST_RESULT
    if "nc" not in _NC_CACHE:
        _NC_CACHE["nc"] = build_nc(T_FULL)
    nc = _NC_CACHE["nc"]
    in_maps = make_in_maps(x, w_attn, b_attn, w_proj, b_proj)
    res = bass_utils.run_bass_kernel_spmd(
        nc, in_maps, core_ids=list(range(N_CORES)), trace=TRACE)
    LAST_RESULT = res
    B, T, C = np.asarray(x).shape
    # host-folded bias: v-bias commutes through softmax (weights sum to 1),
    # so y += (b_attn_v @ w_proj) + b_proj once per row.
    b_attn = np.asarray(b_attn, np.float32)
    hbias = b_attn[2 * C:3 * C] @ np.asarray(w_proj, np.float32) \
        + np.asarray(b_proj, np.float32)
    y = np.empty((B, T, C), np.float32)
    for b in range(B):
        y[b] = res.results[2 * b]["y"].astype(np.float32) \
            + res.results[2 * b + 1]["y"].astype(np.float32) + hbias
    return y
